# revision 8
# baseline (speedup 1.0000x reference)
"""Coupled 2-population RNN on 8 Trainium2 NeuronCores.

Strategy: data-parallel over batch (32 samples/core, weights replicated).
Per step, the combined update  tcat = Wfull @ hcat + Win_aug @ xaug  is one
block matmul with the weights as the PE stationary operand and the state
(feature-major, batch on the free dim) as the moving operand, accumulating
in a single PSUM bank.  tanh runs on ScalarE, the leaky blend on VectorE.
The output projection (combined @ W_out.T) runs as a dense GEMM after the
scan, reading the stored states back from DRAM.

Self-contained: hardcodes all shapes; builds/compiles the Bass program on
first call and runs it via run_bass_kernel_spmd on cores 0..7.
"""

import os
import sys

import numpy as np

sys.path.insert(0, "/opt/trn_rl_repo")

B, T, I = 256, 200, 2
P1, P2, O = 1024, 512, 512
A1, A2 = 0.1, 0.05
NCORES = 8
BL = B // NCORES            # 32 batch per core
PC = P1 + P2                # 1536 combined features
KT = PC // 128              # 12 k-blocks (contraction tiles)
MT = PC // 128              # 12 m-tiles (output tiles)
M1 = P1 // 128              # 8 tiles belong to pop1
C1 = M1 * BL                # 256 state cols for pop1
OT = O // 128               # 4 output-projection m-tiles
NB = 2                      # batches per projection chunk
NF = NB * T                 # 400 moving cols per projection matmul
NCH = BL // NB              # 16 projection chunks

_CACHE = {}


def _emit(tc, nc, mybir, T_steps, repeats=1):
    f32 = mybir.dt.float32
    Tanh = mybir.ActivationFunctionType.Tanh
    MUL = mybir.AluOpType.mult
    ADD = mybir.AluOpType.add

    d = {n: nc.dram_tensor(n, s, f32, kind=k).ap() for n, s, k in [
        ("Wsb", [128, KT * PC], "ExternalInput"),
        ("Wx", [4, PC], "ExternalInput"),
        ("WoT", [128, KT * O], "ExternalInput"),
        ("xaugT", [4, T * BL], "ExternalInput"),
        ("h0T", [128, MT * BL], "ExternalInput"),
        ("h1s", [BL, T, P1], "ExternalOutput"),
        ("h2s", [BL, T, P2], "ExternalOutput"),
        ("outs", [BL, T, O], "ExternalOutput"),
    ]}

    with tc.tile_pool(name="wts", bufs=1) as wpool:
        Wsb = wpool.tile([128, KT * PC], f32)
        Wx = wpool.tile([4, PC], f32)
        WoT = wpool.tile([128, KT * O], f32)
        xa = wpool.tile([4, T * BL], f32)
        nc.sync.dma_start(Wsb[:], d["Wsb"])
        nc.sync.dma_start(Wx[:], d["Wx"])
        nc.sync.dma_start(WoT[:], d["WoT"])
        nc.sync.dma_start(xa[:], d["xaugT"])

        for _rep in range(repeats):
            _emit_body(tc, nc, mybir, T_steps, d, Wsb, Wx, WoT, xa)


def _emit_body(tc, nc, mybir, T_steps, d, Wsb, Wx, WoT, xa):
    f32 = mybir.dt.float32
    Tanh = mybir.ActivationFunctionType.Tanh
    MUL = mybir.AluOpType.mult
    ADD = mybir.AluOpType.add
    if True:
        with (
            tc.tile_pool(name="state", bufs=2) as spool,
            tc.tile_pool(name="work", bufs=2) as wk,
            tc.tile_pool(name="psum", bufs=2, space="PSUM") as pp,
        ):
            h = spool.tile([128, MT * BL], f32, tag="h")
            nc.sync.dma_start(h[:], d["h0T"])

            for t in range(T_steps):
                ps = pp.tile([128, MT * BL], f32, tag="ps")
                th = wk.tile([128, MT * BL], f32, tag="th")
                hn = spool.tile([128, MT * BL], f32, tag="h")
                for m in range(MT):
                    oslc = ps[:, m * BL:(m + 1) * BL]
                    nc.tensor.matmul(
                        oslc,
                        Wx[0:4, m * 128:(m + 1) * 128],
                        xa[0:4, t * BL:(t + 1) * BL],
                        start=True, stop=False,
                    )
                    for k in range(KT):
                        nc.tensor.matmul(
                            oslc,
                            Wsb[:, k * PC + m * 128: k * PC + (m + 1) * 128],
                            h[:, k * BL:(k + 1) * BL],
                            start=False, stop=(k == KT - 1),
                        )
                    nc.scalar.activation(
                        th[:, m * BL:(m + 1) * BL], oslc, Tanh)
                d1 = wk.tile([128, C1], f32, tag="d1")
                nc.vector.scalar_tensor_tensor(
                    d1[:], h[:, 0:C1], -1.0, th[:, 0:C1], MUL, ADD)
                nc.vector.scalar_tensor_tensor(
                    hn[:, 0:C1], d1[:], A1, h[:, 0:C1], MUL, ADD)
                d2 = wk.tile([128, MT * BL - C1], f32, tag="d2")
                nc.vector.scalar_tensor_tensor(
                    d2[:], h[:, C1:], -1.0, th[:, C1:], MUL, ADD)
                nc.vector.scalar_tensor_tensor(
                    hn[:, C1:], d2[:], A2, h[:, C1:], MUL, ADD)
                for m in range(MT):
                    dst = d["h1s"] if m < M1 else d["h2s"]
                    moff = (m if m < M1 else m - M1) * 128
                    nc.sync.dma_start(
                        dst[:, t, moff:moff + 128].transpose([1, 0]),
                        hn[:, m * BL:(m + 1) * BL])
                h = hn

        # Output projection: outs[b,t,o] = sum_q W_out[o,q] * hcat[b,t,q]
        with (
            tc.tile_pool(name="proj", bufs=1) as ppool,
            tc.tile_pool(name="ppsum", bufs=4, space="PSUM") as pps,
            tc.tile_pool(name="pout", bufs=4) as opool,
        ):
            for n in range(NCH):
                rts = []
                for k in range(KT):
                    rt = ppool.tile([128, NF], f32, tag=f"rhs{k}", bufs=2)
                    src = d["h1s"] if k < M1 else d["h2s"]
                    koff = (k if k < M1 else k - M1) * 128
                    for b in range(NB):
                        nc.sync.dma_start(
                            rt[:, b * T:(b + 1) * T],
                            src[n * NB + b, :, koff:koff + 128]
                            .transpose([1, 0]))
                    rts.append(rt)
                for mo in range(OT):
                    pso = pps.tile([128, NF], f32, tag="pso")
                    for k in range(KT):
                        nc.tensor.matmul(
                            pso[:],
                            WoT[:, k * O + mo * 128: k * O + (mo + 1) * 128],
                            rts[k][:],
                            start=(k == 0), stop=(k == KT - 1),
                        )
                    ot = opool.tile([128, NF], f32, tag="ot")
                    nc.scalar.copy(ot[:], pso[:])
                    for b in range(NB):
                        nc.sync.dma_start(
                            d["outs"][n * NB + b, :, mo * 128:(mo + 1) * 128]
                            .transpose([1, 0]),
                            ot[:, b * T:(b + 1) * T])


def _build(T_steps=T, repeats=1):
    from concourse import bacc, tile
    from concourse import mybir

    nc = bacc.Bacc("TRN2", target_bir_lowering=False, debug=False,
                   num_devices=NCORES)
    with tile.TileContext(nc) as tc:
        _emit(tc, nc, mybir, T_steps, repeats)
    nc.compile()
    return nc


def _pack_host(inputs, place_cells_0, W_in1, b_in1, W_in2, b_in2,
               W_rec1, b_rec1, W_rec2, b_rec2, W_12, b_12, W_21, b_21,
               W_out, W_h1, W_h2):
    f = np.float32
    Wfull = np.block([[W_rec1, W_21], [W_12, W_rec2]]).astype(f)
    Wsb = np.ascontiguousarray(
        Wfull.T.reshape(KT, 128, PC).transpose(1, 0, 2).reshape(128, KT * PC))
    Wincat = np.concatenate([W_in1, W_in2], axis=0).astype(f)      # (1536, 2)
    bcat = np.concatenate([b_in1 + b_rec1 + b_21,
                           b_in2 + b_rec2 + b_12]).astype(f)       # (1536,)
    Wx = np.zeros((4, PC), f)
    Wx[0:2] = Wincat.T
    Wx[2] = bcat
    WoT = np.ascontiguousarray(
        W_out.T.reshape(KT, 128, O).transpose(1, 0, 2).reshape(128, KT * O)
    ).astype(f)
    h0cat = np.concatenate(
        [place_cells_0 @ W_h1.T, place_cells_0 @ W_h2.T], axis=1).astype(f)

    in_maps = []
    for c in range(NCORES):
        b0 = c * BL
        xaug = np.zeros((4, T, BL), f)
        xaug[0:2] = inputs[b0:b0 + BL].transpose(2, 1, 0)
        xaug[2] = 1.0
        h0T = np.ascontiguousarray(
            h0cat[b0:b0 + BL].T.reshape(MT, 128, BL).transpose(1, 0, 2)
            .reshape(128, MT * BL))
        in_maps.append({
            "Wsb": Wsb, "Wx": Wx, "WoT": WoT,
            "xaugT": np.ascontiguousarray(xaug.reshape(4, T * BL)),
            "h0T": h0T,
        })
    return in_maps


def kernel(**inputs):
    from concourse.bass_utils import run_bass_kernel_spmd

    if "nc" not in _CACHE:
        _CACHE["nc"] = _build()
    nc = _CACHE["nc"]

    in_maps = _pack_host(**inputs)
    res = run_bass_kernel_spmd(nc, in_maps, core_ids=list(range(NCORES)))
    _CACHE["last_results"] = res

    f = np.float32
    h1s = np.empty((B, T, P1), f)
    h2s = np.empty((B, T, P2), f)
    outs = np.empty((B, T, O), f)
    for c in range(NCORES):
        r = res.results[c]
        h1s[c * BL:(c + 1) * BL] = r["h1s"]
        h2s[c * BL:(c + 1) * BL] = r["h2s"]
        outs[c * BL:(c + 1) * BL] = r["outs"]
    return h1s, h2s, outs
